# revision 17
# baseline (speedup 1.0000x reference)
"""Trainium2 Bass kernel for an AttentionBlock (GroupNorm -> 1x1 qkv conv ->
multi-head self-attention -> 1x1 proj conv -> residual).

Full-input contract: kernel(**inputs) takes the complete unsharded inputs and
returns the complete output. Internally the batch dimension (B=8) is
data-parallel across the 8 NeuronCores: core b processes batch element b
end-to-end (no collectives needed).

Per-core layout strategy (all hardcoded for B=8, C=512, H=W=32 -> S=1024,
HEADS=8, D=64, GROUPS=32):
  - x is held channel-major in SBUF as [128 part, 4 blk, 1024 s].
  - GroupNorm: bn_stats/bn_aggr per channel, then tiny matmuls against a
    block-diagonal (1/16) matrix reduce the 16 channels of each group across
    partitions; rsqrt built as exp(-0.5*ln(var+eps)) so only one ACT table set
    (natural_log_exp) is ever loaded.
  - q,k are produced channel-major (needs W_qkv^T which is built once with PE
    transposes); v is produced directly spatial-major ([s, 64] per head) by
    contracting xn as lhsT, so no per-head transposes are needed.
  - Attention per head computes logits^T (s on partitions, t free) so softmax's
    normalization can ride the PV matmul: a ones-column appended to v^T yields
    the softmax denominators in PSUM row 64 for free. exp() runs on ScalarE
    straight out of PSUM with the 1/sqrt(D) scale folded into ACT's free
    affine; max-subtraction is skipped (logits are bounded ~ +-8, safe in f32).
  - Head pairs live in partition halves (base partition 0/64) so the K=64
    logits matmuls of a pair can run concurrently in separate PE row groups.
  - proj + bias + residual: matmul, then one DVE pass adding (x + b_proj).
"""

import numpy as np

B, C, S = 8, 512, 1024
HH, WW = 32, 32
HEADS, D, GROUPS, GS = 8, 64, 32, 16
EPS = 1e-5
P = 128
NCB = C // P  # 4 channel blocks
NSB = S // P  # 8 spatial blocks
N_CORES = 8

_CACHE = {}


def _build_program(debug_outs=False):
    import concourse.bacc as bacc
    import concourse.bass as bass
    import concourse.mybir as mybir
    import concourse.tile as tile
    from concourse.masks import make_identity

    f32 = mybir.dt.float32
    bf16 = mybir.dt.bfloat16
    AF = mybir.ActivationFunctionType
    OP = mybir.AluOpType

    nc = bacc.Bacc()

    x_d = nc.dram_tensor("x", [C, S], f32, kind="ExternalInput")
    gamma_d = nc.dram_tensor("gamma", [C], f32, kind="ExternalInput")
    beta_d = nc.dram_tensor("beta", [C], f32, kind="ExternalInput")
    wqkv_d = nc.dram_tensor("w_qkv", [3 * C, C], f32, kind="ExternalInput")
    bqkv_d = nc.dram_tensor("b_qkv", [3 * C], f32, kind="ExternalInput")
    wproj_d = nc.dram_tensor("w_proj", [C, C], f32, kind="ExternalInput")
    bproj_d = nc.dram_tensor("b_proj", [C], f32, kind="ExternalInput")
    out_d = nc.dram_tensor("out", [C, S], f32, kind="ExternalOutput")

    x_blk = x_d.rearrange("(blk p) s -> p blk s", p=P)        # [128, 4, 1024]
    out_blk = out_d.rearrange("(blk p) s -> p blk s", p=P)    # [128, 4, 1024]

    with tile.TileContext(nc) as tc:
        with (
            tc.tile_pool(name="big", bufs=1) as big,
            tc.tile_pool(name="wstage", bufs=4) as wstage,
            tc.tile_pool(name="small", bufs=4) as small,
            tc.tile_pool(name="exps", bufs=2) as exps,
            tc.tile_pool(name="psl", bufs=2, space="PSUM") as psl,
            tc.tile_pool(name="pso", bufs=2, space="PSUM") as pso,
            tc.tile_pool(name="dpool", bufs=2, space="DRAM") as dpool,
        ):
            # ---------------- input DMAs ----------------
            x_sb = big.tile([P, NCB, S], f32, tag="x_sb")
            for blk in range(NCB):
                nc.sync.dma_start(out=x_sb[:, blk, :], in_=x_blk[:, blk, :])

            gamma_sb = big.tile([P, NCB], f32, tag="gamma_sb")
            nc.sync.dma_start(out=gamma_sb, in_=gamma_d.rearrange("(blk p) -> p blk", p=P))
            beta_sb = big.tile([P, NCB], f32, tag="beta_sb")
            nc.sync.dma_start(out=beta_sb, in_=beta_d.rearrange("(blk p) -> p blk", p=P))
            bqkv_sb = big.tile([P, 12], f32, tag="bqkv_sb")
            nc.sync.dma_start(out=bqkv_sb, in_=bqkv_d.rearrange("(blk p) -> p blk", p=P))
            bproj_sb = big.tile([P, NCB], f32, tag="bproj_sb")
            nc.sync.dma_start(out=bproj_sb, in_=bproj_d.rearrange("(blk p) -> p blk", p=P))
            # b_v replicated across all partitions (DMA can broadcast, DVE can't)
            bv_rep = big.tile([P, HEADS, D], f32, tag="bv_rep")
            bv_src = bqkv_d[2 * C:3 * C]
            nc.sync.dma_start(
                out=bv_rep,
                in_=bass.AP(
                    tensor=bv_src.tensor,
                    offset=bv_src.offset,
                    ap=[[0, P], [D, HEADS], [1, D]],
                ),
            )

            # ---------------- constants ----------------
            ident = big.tile([P, P], f32, tag="ident")
            make_identity(nc, ident)
            eps_sb = big.tile([P, 1], f32, tag="eps_sb")
            nc.gpsimd.memset(eps_sb, EPS)
            # gmat[c, g] = 1/(16*S) if channel c (within a 128-block) is in
            # local group g else 0; gmat2 = indicator transpose (broadcast
            # back). Built with affine_select (out = cond ? in : fill) since
            # memsets on 16-partition slices violate the 32-align rule.
            gmat = big.tile([P, 8], f32, tag="gmat")
            nc.gpsimd.memset(gmat, 0.0)
            nc.gpsimd.affine_select(  # fill 1/16 where c-16g-15 <= 0
                out=gmat, in_=gmat, compare_op=OP.is_gt,
                fill=1.0 / GS, base=-15, pattern=[[-GS, 8]],
                channel_multiplier=1,
            )
            nc.gpsimd.affine_select(  # zero where c-16g < 0
                out=gmat, in_=gmat, compare_op=OP.is_ge,
                fill=0.0, base=0, pattern=[[-GS, 8]],
                channel_multiplier=1,
            )
            gmat2 = big.tile([8, P], f32, tag="gmat2")
            nc.gpsimd.memset(gmat2, 0.0)
            nc.gpsimd.affine_select(
                out=gmat2, in_=gmat2, compare_op=OP.is_gt,
                fill=1.0, base=-15, pattern=[[1, P]],
                channel_multiplier=-GS,
            )
            nc.gpsimd.affine_select(
                out=gmat2, in_=gmat2, compare_op=OP.is_ge,
                fill=0.0, base=0, pattern=[[1, P]],
                channel_multiplier=-GS,
            )

            # ---------------- W^T via PE transposes ----------------
            wqkvT = big.tile([P, NCB, 3 * C], f32, tag="wqkvT")  # [i, iblk, o]
            for m in range(12):  # source row blocks of w_qkv (out channels)
                w_raw = wstage.tile([P, C], f32, tag="w_raw")
                nc.sync.dma_start(
                    out=w_raw,
                    in_=wqkv_d.rearrange("(mb p) i -> p mb i", p=P)[:, m, :],
                )
                for dblk in range(NCB):
                    pt = pso.tile([P, 1024], f32, tag="pso")
                    nc.tensor.transpose(pt[:, 0:P], w_raw[:, dblk * P:(dblk + 1) * P], ident)
                    nc.scalar.copy(
                        out=wqkvT[:, dblk, m * P:(m + 1) * P], in_=pt[:, 0:P]
                    )
            wprojT = big.tile([P, NCB, C], f32, tag="wprojT")
            for m in range(NCB):
                w_raw = wstage.tile([P, C], f32, tag="w_raw")
                nc.sync.dma_start(
                    out=w_raw,
                    in_=wproj_d.rearrange("(mb p) i -> p mb i", p=P)[:, m, :],
                )
                for dblk in range(NCB):
                    pt = pso.tile([P, 1024], f32, tag="pso")
                    nc.tensor.transpose(pt[:, 0:P], w_raw[:, dblk * P:(dblk + 1) * P], ident)
                    nc.scalar.copy(
                        out=wprojT[:, dblk, m * P:(m + 1) * P], in_=pt[:, 0:P]
                    )

            # ---------------- GroupNorm ----------------
            xn = big.tile([P, NCB, S], f32, tag="xn")
            for blk in range(NCB):
                stats = small.tile([P, 2, 6], f32, tag="stats")
                nc.vector.bn_stats(out=stats[:, 0, :], in_=x_sb[:, blk, 0:512])
                nc.vector.bn_stats(out=stats[:, 1, :], in_=x_sb[:, blk, 512:1024])
                mv = small.tile([P, 2], f32, tag="mv")
                nc.vector.bn_aggr(out=mv, in_=stats)
                # t2 = [sum-contrib mean, var + mean^2] per channel
                t2 = small.tile([P, 2], f32, tag="t2")
                nc.vector.tensor_copy(t2[:, 0:1], mv[:, 0:1])
                sq = small.tile([P, 1], f32, tag="sq")
                nc.vector.tensor_mul(sq, mv[:, 0:1], mv[:, 0:1])
                nc.vector.tensor_add(t2[:, 1:2], mv[:, 1:2], sq)
                # cross-partition group reduce: psg[g, :] = [mu_g, E2_g]
                # (bn stats are already per-channel means over S, so the
                #  group combine just averages the 16 channels: gmat = 1/16)
                psg = psl.tile([P, 1024], f32, tag="psl")
                nc.tensor.matmul(psg[0:8, 0:2], gmat, t2, start=True, stop=True)
                # var_g = E2 - mu^2 ; rstd = exp(-0.5*ln(var+eps))
                musr = small.tile([8, 2], f32, tag="musr")
                nc.vector.tensor_copy(musr, psg[0:8, 0:2])  # [mu, E2]
                sqmu = small.tile([8, 1], f32, tag="sqmu")
                nc.vector.tensor_mul(sqmu, musr[:, 0:1], musr[:, 0:1])
                varg = small.tile([8, 1], f32, tag="varg")
                nc.vector.tensor_tensor(varg, musr[:, 1:2], sqmu, OP.subtract)
                lnv = small.tile([8, 1], f32, tag="lnv")
                nc.scalar.activation(lnv, varg, AF.Ln, bias=eps_sb[0:8, :], scale=1.0)
                nc.scalar.activation(musr[:, 1:2], lnv, AF.Exp, scale=-0.5)
                # broadcast (mu, rstd) back to the 128 channels of this block
                psb = psl.tile([P, 1024], f32, tag="psl")
                nc.tensor.matmul(psb[:, 0:2], gmat2, musr, start=True, stop=True)
                scale_c = small.tile([P, 1], f32, tag="scale_c")
                nc.vector.tensor_mul(scale_c, psb[:, 1:2], gamma_sb[:, blk:blk + 1])
                nm = small.tile([P, 1], f32, tag="nm")
                nc.vector.tensor_mul(nm, psb[:, 0:1], scale_c)
                bias_c = small.tile([P, 1], f32, tag="bias_c")
                nc.vector.tensor_tensor(bias_c, beta_sb[:, blk:blk + 1], nm, OP.subtract)
                nc.vector.tensor_scalar(
                    xn[:, blk, :], x_sb[:, blk, :], scale_c, bias_c, OP.mult, OP.add
                )

            # xpb = x + b_proj (residual + proj bias), in place over x_sb
            for blk in range(NCB):
                nc.vector.tensor_scalar(
                    x_sb[:, blk, :], x_sb[:, blk, :], bproj_sb[:, blk:blk + 1], None,
                    OP.add,
                )

            # ---------------- qk (channel-major) ----------------
            # qk_sb blocks 0..3 = q channels 0..511, blocks 4..7 = k channels
            qk_sb = big.tile([P, 8, S], f32, tag="qk_sb")
            for m in range(8):
                pq = psl.tile([P, 1024], f32, tag="psl")
                for n in range(2):
                    for kk in range(NCB):
                        nc.tensor.matmul(
                            pq[:, n * 512:(n + 1) * 512],
                            wqkvT[:, kk, m * P:(m + 1) * P],
                            xn[:, kk, n * 512:(n + 1) * 512],
                            start=(kk == 0), stop=(kk == NCB - 1),
                        )
                nc.vector.tensor_scalar(
                    qk_sb[:, m, :], pq, bqkv_sb[:, m:m + 1], None, OP.add
                )

            # ---------------- v^T (spatial-major, with ones column) --------
            vT = big.tile([P, NSB, HEADS, D + 1], bf16, tag="vT")
            for sblk in range(NSB):
                pv = pso.tile([P, 1024], f32, tag="pso")
                for kk in range(NCB):
                    nc.tensor.matmul(
                        pv[:, 0:512],
                        xn[:, kk, sblk * P:(sblk + 1) * P],
                        wqkvT[:, kk, 2 * C:3 * C],
                        start=(kk == 0), stop=(kk == NCB - 1),
                    )
                nc.vector.tensor_tensor(
                    vT[:, sblk, :, 0:D],
                    pv[:, 0:512].rearrange("p (h d) -> p h d", h=HEADS),
                    bv_rep,
                    OP.add,
                )
                nc.gpsimd.memset(vT[:, sblk, :, D:D + 1], 1.0)

            # ---------------- attention ----------------
            hout = big.tile([P, NCB, S], f32, tag="hout")
            s2 = 1.0 / float(np.sqrt(D))  # folded 1/sqrt(sqrt(D)) on q and k
            for hp in range(4):  # head pairs -> PE row-group concurrency
                explts = []
                for h2 in range(2):
                    h = 2 * hp + h2
                    p0 = (h % 2) * D
                    explt = exps.tile([P, NSB, S], bf16, tag="explt")
                    explts.append(explt)
                    for sblk in range(NSB):
                        pl = psl.tile([P, 1024], f32, tag="psl")
                        for n in range(2):
                            nc.tensor.matmul(
                                pl[:, n * 512:(n + 1) * 512],
                                qk_sb[p0:p0 + D, 4 + h // 2, sblk * P:(sblk + 1) * P],
                                qk_sb[p0:p0 + D, h // 2, n * 512:(n + 1) * 512],
                                start=True, stop=True,
                            )
                        nc.scalar.activation(
                            explt[:, sblk, :], pl, AF.Exp, scale=s2
                        )
                for h2 in range(2):
                    h = 2 * hp + h2
                    explt = explts[h2]
                    po = pso.tile([P, 1024], f32, tag="pso")
                    for n in range(2):
                        for sblk in range(NSB):
                            nc.tensor.matmul(
                                po[0:D + 1, n * 512:(n + 1) * 512],
                                vT[:, sblk, h, :],
                                explt[:, sblk, n * 512:(n + 1) * 512],
                                start=(sblk == 0), stop=(sblk == NSB - 1),
                            )
                    rinv = small.tile([1, S], f32, tag="rinv")
                    nc.vector.reciprocal(rinv, po[D:D + 1, :])
                    # replicate rinv across 64 partitions via a DRAM bounce
                    # (neither DVE nor SBUF-source DMA can partition-broadcast)
                    rdram = dpool.tile([S], f32, tag="rdram")
                    nc.sync.dma_start(out=rdram[None, :], in_=rinv)
                    rinv_rep = small.tile([D, S], f32, tag="rinv_rep")
                    nc.sync.dma_start(
                        out=rinv_rep, in_=rdram[None, :].partition_broadcast(D)
                    )
                    nc.vector.tensor_tensor(
                        hout[(h % 2) * D:(h % 2) * D + D, h // 2, :],
                        po[0:D, :],
                        rinv_rep,
                        OP.mult,
                    )

            if debug_outs:
                dbg = {
                    "dbg_xn": (xn, f32), "dbg_qk": (qk_sb, f32),
                    "dbg_vt": (vT, bf16), "dbg_hout": (hout, f32),
                }
                for name, (tile_ap, dt) in dbg.items():
                    od = nc.dram_tensor(name, list(tile_ap.shape), dt,
                                        kind="ExternalOutput")
                    nc.sync.dma_start(out=od[:], in_=tile_ap[:])

            # ---------------- proj + residual ----------------
            for m in range(NCB):
                pp = pso.tile([P, 1024], f32, tag="pso")
                for n in range(2):
                    for kk in range(NCB):
                        nc.tensor.matmul(
                            pp[:, n * 512:(n + 1) * 512],
                            wprojT[:, kk, m * P:(m + 1) * P],
                            hout[:, kk, n * 512:(n + 1) * 512],
                            start=(kk == 0), stop=(kk == NCB - 1),
                        )
                nc.vector.tensor_tensor(xn[:, m, :], pp, x_sb[:, m, :], OP.add)
                nc.sync.dma_start(out=out_blk[:, m, :], in_=xn[:, m, :])

    # Bacc defers register allocation etc. to its compile pass; the
    # run_bass_via_pjrt path serializes the module as-is, so run it here.
    nc.finalize()
    return nc


def _get_program():
    if "nc" not in _CACHE:
        _CACHE["nc"] = _build_program()
    return _CACHE["nc"]


def kernel(x, gamma, beta, w_qkv, b_qkv, w_proj, b_proj, **run_kwargs):
    from concourse import bass_utils

    nc = _get_program()
    in_maps = []
    for b in range(B):
        in_maps.append({
            "x": np.ascontiguousarray(x[b].reshape(C, S), dtype=np.float32),
            "gamma": np.ascontiguousarray(gamma, dtype=np.float32),
            "beta": np.ascontiguousarray(beta, dtype=np.float32),
            "w_qkv": np.ascontiguousarray(w_qkv, dtype=np.float32),
            "b_qkv": np.ascontiguousarray(b_qkv, dtype=np.float32),
            "w_proj": np.ascontiguousarray(w_proj, dtype=np.float32),
            "b_proj": np.ascontiguousarray(b_proj, dtype=np.float32),
        })
    res = bass_utils.run_bass_kernel_spmd(
        nc, in_maps, core_ids=list(range(N_CORES)), **run_kwargs
    )
    out = np.stack(
        [res.results[b]["out"].reshape(C, HH, WW) for b in range(B)], axis=0
    )
    _CACHE["last_result"] = res
    return out


# revision 24
# speedup vs baseline: 1.2337x; 1.2337x over previous
"""Trainium2 Bass kernel for an AttentionBlock (GroupNorm -> 1x1 qkv conv ->
multi-head self-attention -> 1x1 proj conv -> residual).

Full-input contract: kernel(**inputs) takes the complete unsharded inputs and
returns the complete output. Internally the batch dimension (B=8) is
data-parallel across the 8 NeuronCores: core b processes batch element b
end-to-end (no collectives needed).

Per-core layout strategy (all hardcoded for B=8, C=512, H=W=32 -> S=1024,
HEADS=8, D=64, GROUPS=32):
  - x is held channel-major in SBUF as [128 part, 4 blk, 1024 s].
  - GroupNorm: bn_stats/bn_aggr per channel, then tiny matmuls against a
    block-diagonal (1/16) matrix reduce the 16 channels of each group across
    partitions; rsqrt built as exp(-0.5*ln(var+eps)) so only one ACT table set
    (natural_log_exp) is ever loaded.
  - q,k are produced channel-major (needs W_qkv^T which is built once with PE
    transposes); v is produced directly spatial-major ([s, 64] per head) by
    contracting xn as lhsT, so no per-head transposes are needed.
  - Attention per head computes logits^T (s on partitions, t free) so softmax's
    normalization can ride the PV matmul: a ones-column appended to v^T yields
    the softmax denominators in PSUM row 64 for free. exp() runs on ScalarE
    straight out of PSUM with the 1/sqrt(D) scale folded into ACT's free
    affine; max-subtraction is skipped (logits are bounded ~ +-8, safe in f32).
  - Head pairs live in partition halves (base partition 0/64) so the K=64
    logits matmuls of a pair can run concurrently in separate PE row groups.
  - proj + bias + residual: matmul, then one DVE pass adding (x + b_proj).
"""

import numpy as np

B, C, S = 8, 512, 1024
HH, WW = 32, 32
HEADS, D, GROUPS, GS = 8, 64, 32, 16
EPS = 1e-5
P = 128
NCB = C // P  # 4 channel blocks
NSB = S // P  # 8 spatial blocks
N_CORES = 8

_CACHE = {}


def _build_program(debug_outs=False):
    import concourse.bacc as bacc
    import concourse.bass as bass
    import concourse.mybir as mybir
    import concourse.tile as tile
    from concourse.masks import make_identity

    f32 = mybir.dt.float32
    bf16 = mybir.dt.bfloat16
    AF = mybir.ActivationFunctionType
    OP = mybir.AluOpType

    nc = bacc.Bacc()

    x_d = nc.dram_tensor("x", [C, S], f32, kind="ExternalInput")
    gamma_d = nc.dram_tensor("gamma", [C], f32, kind="ExternalInput")
    beta_d = nc.dram_tensor("beta", [C], f32, kind="ExternalInput")
    wqkv_d = nc.dram_tensor("w_qkv", [3 * C, C], f32, kind="ExternalInput")
    bqkv_d = nc.dram_tensor("b_qkv", [3 * C], f32, kind="ExternalInput")
    wproj_d = nc.dram_tensor("w_proj", [C, C], f32, kind="ExternalInput")
    bproj_d = nc.dram_tensor("b_proj", [C], f32, kind="ExternalInput")
    out_d = nc.dram_tensor("out", [C, S], f32, kind="ExternalOutput")

    x_blk = x_d.rearrange("(blk p) s -> p blk s", p=P)        # [128, 4, 1024]
    out_blk = out_d.rearrange("(blk p) s -> p blk s", p=P)    # [128, 4, 1024]

    with tile.TileContext(nc) as tc:
        with (
            tc.tile_pool(name="big", bufs=1) as big,
            tc.tile_pool(name="wstage", bufs=4) as wstage,
            tc.tile_pool(name="small", bufs=4) as small,
            tc.tile_pool(name="exps", bufs=2) as exps,
            tc.tile_pool(name="psl", bufs=2, space="PSUM") as psl,
            tc.tile_pool(name="pso", bufs=2, space="PSUM") as pso,
            tc.tile_pool(name="dpool", bufs=2, space="DRAM") as dpool,
        ):
            # ---------------- input DMAs ----------------
            x_sb = big.tile([P, NCB, S], f32, tag="x_sb")
            for blk in range(NCB):
                nc.sync.dma_start(out=x_sb[:, blk, :], in_=x_blk[:, blk, :])

            gamma_sb = big.tile([P, NCB], f32, tag="gamma_sb")
            nc.sync.dma_start(out=gamma_sb, in_=gamma_d.rearrange("(blk p) -> p blk", p=P))
            beta_sb = big.tile([P, NCB], f32, tag="beta_sb")
            nc.sync.dma_start(out=beta_sb, in_=beta_d.rearrange("(blk p) -> p blk", p=P))
            bqkv_sb = big.tile([P, 12], f32, tag="bqkv_sb")
            nc.sync.dma_start(out=bqkv_sb, in_=bqkv_d.rearrange("(blk p) -> p blk", p=P))
            bproj_sb = big.tile([P, NCB], f32, tag="bproj_sb")
            nc.sync.dma_start(out=bproj_sb, in_=bproj_d.rearrange("(blk p) -> p blk", p=P))
            # b_v replicated across all partitions (DMA can broadcast, DVE can't)
            bv_rep = big.tile([P, HEADS, D], f32, tag="bv_rep")
            bv_src = bqkv_d[2 * C:3 * C]
            nc.sync.dma_start(
                out=bv_rep,
                in_=bass.AP(
                    tensor=bv_src.tensor,
                    offset=bv_src.offset,
                    ap=[[0, P], [D, HEADS], [1, D]],
                ),
            )

            # ---------------- constants ----------------
            ident = big.tile([P, P], f32, tag="ident")
            make_identity(nc, ident)
            eps_sb = big.tile([P, 1], f32, tag="eps_sb")
            nc.gpsimd.memset(eps_sb, EPS)
            # gmat[c, g] = 1/(16*S) if channel c (within a 128-block) is in
            # local group g else 0; gmat2 = indicator transpose (broadcast
            # back). Built with affine_select (out = cond ? in : fill) since
            # memsets on 16-partition slices violate the 32-align rule.
            gmat = big.tile([P, 8], f32, tag="gmat")
            nc.gpsimd.memset(gmat, 0.0)
            nc.gpsimd.affine_select(  # fill 1/16 where c-16g-15 <= 0
                out=gmat, in_=gmat, compare_op=OP.is_gt,
                fill=1.0 / GS, base=-15, pattern=[[-GS, 8]],
                channel_multiplier=1,
            )
            nc.gpsimd.affine_select(  # zero where c-16g < 0
                out=gmat, in_=gmat, compare_op=OP.is_ge,
                fill=0.0, base=0, pattern=[[-GS, 8]],
                channel_multiplier=1,
            )
            gmat2 = big.tile([8, P], f32, tag="gmat2")
            nc.gpsimd.memset(gmat2, 0.0)
            nc.gpsimd.affine_select(
                out=gmat2, in_=gmat2, compare_op=OP.is_gt,
                fill=1.0, base=-15, pattern=[[1, P]],
                channel_multiplier=-GS,
            )
            nc.gpsimd.affine_select(
                out=gmat2, in_=gmat2, compare_op=OP.is_ge,
                fill=0.0, base=0, pattern=[[1, P]],
                channel_multiplier=-GS,
            )

            # ---------------- W^T via PE transposes ----------------
            # stored bf16: fp32 matmuls run fp32_mode=LOW_HIGH (two passes,
            # ~2x slower), bf16 operands stream at full rate
            wqkvT = big.tile([P, NCB, 3 * C], bf16, tag="wqkvT")  # [i, iblk, o]
            for m in range(12):  # source row blocks of w_qkv (out channels)
                w_raw = wstage.tile([P, C], f32, tag="w_raw")
                nc.sync.dma_start(
                    out=w_raw,
                    in_=wqkv_d.rearrange("(mb p) i -> p mb i", p=P)[:, m, :],
                )
                for dblk in range(NCB):
                    pt = pso.tile([P, 1024], f32, tag="pso")
                    nc.tensor.transpose(pt[:, 0:P], w_raw[:, dblk * P:(dblk + 1) * P], ident)
                    nc.scalar.copy(
                        out=wqkvT[:, dblk, m * P:(m + 1) * P], in_=pt[:, 0:P]
                    )
            wprojT = big.tile([P, NCB, C], bf16, tag="wprojT")
            for m in range(NCB):
                w_raw = wstage.tile([P, C], f32, tag="w_raw")
                nc.sync.dma_start(
                    out=w_raw,
                    in_=wproj_d.rearrange("(mb p) i -> p mb i", p=P)[:, m, :],
                )
                for dblk in range(NCB):
                    pt = pso.tile([P, 1024], f32, tag="pso")
                    nc.tensor.transpose(pt[:, 0:P], w_raw[:, dblk * P:(dblk + 1) * P], ident)
                    nc.scalar.copy(
                        out=wprojT[:, dblk, m * P:(m + 1) * P], in_=pt[:, 0:P]
                    )

            # ---------------- GroupNorm ----------------
            # phase 1: per-channel stats + cross-partition group combine
            xn = big.tile([P, NCB, S], bf16, tag="xn")
            musrs, vargs, lnvs = [], [], []
            for blk in range(NCB):
                stats = small.tile([P, 2, 6], f32, tag="stats")
                nc.vector.bn_stats(out=stats[:, 0, :], in_=x_sb[:, blk, 0:512])
                nc.vector.bn_stats(out=stats[:, 1, :], in_=x_sb[:, blk, 512:1024])
                mv = small.tile([P, 2], f32, tag="mv")
                nc.vector.bn_aggr(out=mv, in_=stats)
                # t2 = [mean, var + mean^2] per channel
                t2 = small.tile([P, 2], f32, tag="t2")
                nc.vector.tensor_copy(t2[:, 0:1], mv[:, 0:1])
                sq = small.tile([P, 1], f32, tag="sq")
                nc.vector.tensor_mul(sq, mv[:, 0:1], mv[:, 0:1])
                nc.vector.tensor_add(t2[:, 1:2], mv[:, 1:2], sq)
                # cross-partition group reduce: psg[g, :] = [mu_g, E2_g]
                # (bn stats are already per-channel means over S, so the
                #  group combine just averages the 16 channels: gmat = 1/16)
                psg = psl.tile([P, 1024], f32, tag="psl")
                nc.tensor.matmul(psg[0:8, 0:2], gmat, t2, start=True, stop=True)
                # var_g = E2 - mu^2 ; rstd = exp(-0.5*ln(var+eps))
                musr = small.tile([8, 2], f32, tag=f"musr{blk}")
                nc.vector.tensor_copy(musr, psg[0:8, 0:2])  # [mu, E2]
                sqmu = small.tile([8, 1], f32, tag="sqmu")
                nc.vector.tensor_mul(sqmu, musr[:, 0:1], musr[:, 0:1])
                varg = small.tile([8, 1], f32, tag=f"varg{blk}")
                nc.vector.tensor_tensor(varg, musr[:, 1:2], sqmu, OP.subtract)
                musrs.append(musr)
                vargs.append(varg)
            # phase 2: batch Ln then Exp so ACT loads each table set once
            for blk in range(NCB):
                lnv = small.tile([8, 1], f32, tag=f"lnv{blk}")
                nc.scalar.activation(lnv, vargs[blk], AF.Ln, bias=eps_sb[0:8, :],
                                     scale=1.0)
                lnvs.append(lnv)
            for blk in range(NCB):
                nc.scalar.activation(musrs[blk][:, 1:2], lnvs[blk], AF.Exp,
                                     scale=-0.5)
            # phase 3: broadcast (mu, rstd) back to channels and normalize
            for blk in range(NCB):
                psb = psl.tile([P, 1024], f32, tag="psl")
                nc.tensor.matmul(psb[:, 0:2], gmat2, musrs[blk], start=True,
                                 stop=True)
                scale_c = small.tile([P, 1], f32, tag="scale_c")
                nc.vector.tensor_mul(scale_c, psb[:, 1:2], gamma_sb[:, blk:blk + 1])
                nm = small.tile([P, 1], f32, tag="nm")
                nc.vector.tensor_mul(nm, psb[:, 0:1], scale_c)
                bias_c = small.tile([P, 1], f32, tag="bias_c")
                nc.vector.tensor_tensor(bias_c, beta_sb[:, blk:blk + 1], nm, OP.subtract)
                nc.vector.tensor_scalar(
                    xn[:, blk, :], x_sb[:, blk, :], scale_c, bias_c, OP.mult, OP.add
                )

            # xpb = x + b_proj (residual + proj bias), in place over x_sb
            for blk in range(NCB):
                nc.vector.tensor_scalar(
                    x_sb[:, blk, :], x_sb[:, blk, :], bproj_sb[:, blk:blk + 1], None,
                    OP.add,
                )

            # ---------------- qk (channel-major) ----------------
            # qk_sb blocks 0..3 = q channels 0..511, blocks 4..7 = k channels
            qk_sb = big.tile([P, 8, S], bf16, tag="qk_sb")
            for m in range(8):
                pq = psl.tile([P, 1024], f32, tag="psl")
                for n in range(2):
                    for kk in range(NCB):
                        nc.tensor.matmul(
                            pq[:, n * 512:(n + 1) * 512],
                            wqkvT[:, kk, m * P:(m + 1) * P],
                            xn[:, kk, n * 512:(n + 1) * 512],
                            start=(kk == 0), stop=(kk == NCB - 1),
                        )
                nc.vector.tensor_scalar(
                    qk_sb[:, m, :], pq, bqkv_sb[:, m:m + 1], None, OP.add
                )

            # ---------------- v^T (spatial-major, with ones column) --------
            vT = big.tile([P, NSB, HEADS, D + 1], bf16, tag="vT")
            for sblk in range(NSB):
                pv = pso.tile([P, 1024], f32, tag="pso")
                for kk in range(NCB):
                    nc.tensor.matmul(
                        pv[:, 0:512],
                        xn[:, kk, sblk * P:(sblk + 1) * P],
                        wqkvT[:, kk, 2 * C:3 * C],
                        start=(kk == 0), stop=(kk == NCB - 1),
                    )
                nc.vector.tensor_tensor(
                    vT[:, sblk, :, 0:D],
                    pv[:, 0:512].rearrange("p (h d) -> p h d", h=HEADS),
                    bv_rep,
                    OP.add,
                )
                nc.gpsimd.memset(vT[:, sblk, :, D:D + 1], 1.0)

            # ---------------- attention ----------------
            hout = big.tile([P, NCB, S], bf16, tag="hout")
            s2 = 1.0 / float(np.sqrt(D))  # folded 1/sqrt(sqrt(D)) on q and k
            for hp in range(4):  # head pairs -> PE row-group concurrency
                explts = []
                for h2 in range(2):
                    h = 2 * hp + h2
                    p0 = (h % 2) * D
                    explt = exps.tile([P, NSB, S], bf16, tag="explt")
                    explts.append(explt)
                    for sblk in range(NSB):
                        pl = psl.tile([P, 1024], f32, tag="psl")
                        for n in range(2):
                            nc.tensor.matmul(
                                pl[:, n * 512:(n + 1) * 512],
                                qk_sb[p0:p0 + D, 4 + h // 2, sblk * P:(sblk + 1) * P],
                                qk_sb[p0:p0 + D, h // 2, n * 512:(n + 1) * 512],
                                start=True, stop=True,
                            )
                        nc.scalar.activation(
                            explt[:, sblk, :], pl, AF.Exp, scale=s2
                        )
                for h2 in range(2):
                    h = 2 * hp + h2
                    explt = explts[h2]
                    po = pso.tile([P, 1024], f32, tag="pso")
                    for n in range(2):
                        for sblk in range(NSB):
                            nc.tensor.matmul(
                                po[0:D + 1, n * 512:(n + 1) * 512],
                                vT[:, sblk, h, :],
                                explt[:, sblk, n * 512:(n + 1) * 512],
                                start=(sblk == 0), stop=(sblk == NSB - 1),
                            )
                    # DVE reciprocal is ~8 cycles/element/lane, so running it
                    # on the [1, S] denominator row uses one lane (~6.5us).
                    # Bounce through DRAM to reshape [S] -> [128, 8], recip on
                    # all lanes (~0.2us), bounce back, then broadcast-load to
                    # the 64 partitions the normalize multiply needs.
                    den = small.tile([1, S], f32, tag="den")
                    nc.vector.tensor_copy(den, po[D:D + 1, :])
                    rd1 = dpool.tile([S], f32, tag="rd1")
                    nc.sync.dma_start(out=rd1[None, :], in_=den)
                    denp = small.tile([P, 8], f32, tag="denp")
                    nc.sync.dma_start(
                        out=denp, in_=rd1.rearrange("(o p) -> p o", p=P)
                    )
                    rinvp = small.tile([P, 8], f32, tag="rinvp")
                    nc.vector.reciprocal(rinvp, denp)
                    rd2 = dpool.tile([S], f32, tag="rd2")
                    nc.sync.dma_start(
                        out=rd2.rearrange("(o p) -> p o", p=P), in_=rinvp
                    )
                    rinv_rep = small.tile([D, S], f32, tag="rinv_rep")
                    nc.sync.dma_start(
                        out=rinv_rep, in_=rd2[None, :].partition_broadcast(D)
                    )
                    nc.vector.tensor_tensor(
                        hout[(h % 2) * D:(h % 2) * D + D, h // 2, :],
                        po[0:D, :],
                        rinv_rep,
                        OP.mult,
                    )

            if debug_outs:
                dbg = {
                    "dbg_xn": (xn, f32), "dbg_qk": (qk_sb, f32),
                    "dbg_vt": (vT, bf16), "dbg_hout": (hout, f32),
                }
                for name, (tile_ap, dt) in dbg.items():
                    od = nc.dram_tensor(name, list(tile_ap.shape), dt,
                                        kind="ExternalOutput")
                    nc.sync.dma_start(out=od[:], in_=tile_ap[:])

            # ---------------- proj + residual ----------------
            out_sb = big.tile([P, NCB, S], f32, tag="out_sb")
            for m in range(NCB):
                pp = pso.tile([P, 1024], f32, tag="pso")
                for n in range(2):
                    for kk in range(NCB):
                        nc.tensor.matmul(
                            pp[:, n * 512:(n + 1) * 512],
                            wprojT[:, kk, m * P:(m + 1) * P],
                            hout[:, kk, n * 512:(n + 1) * 512],
                            start=(kk == 0), stop=(kk == NCB - 1),
                        )
                nc.vector.tensor_tensor(out_sb[:, m, :], pp, x_sb[:, m, :], OP.add)
                nc.sync.dma_start(out=out_blk[:, m, :], in_=out_sb[:, m, :])

    # Bacc defers register allocation etc. to its compile pass; the
    # run_bass_via_pjrt path serializes the module as-is, so run it here.
    nc.finalize()
    return nc


def _get_program():
    if "nc" not in _CACHE:
        _CACHE["nc"] = _build_program()
    return _CACHE["nc"]


def kernel(x, gamma, beta, w_qkv, b_qkv, w_proj, b_proj, **run_kwargs):
    from concourse import bass_utils

    nc = _get_program()
    in_maps = []
    for b in range(B):
        in_maps.append({
            "x": np.ascontiguousarray(x[b].reshape(C, S), dtype=np.float32),
            "gamma": np.ascontiguousarray(gamma, dtype=np.float32),
            "beta": np.ascontiguousarray(beta, dtype=np.float32),
            "w_qkv": np.ascontiguousarray(w_qkv, dtype=np.float32),
            "b_qkv": np.ascontiguousarray(b_qkv, dtype=np.float32),
            "w_proj": np.ascontiguousarray(w_proj, dtype=np.float32),
            "b_proj": np.ascontiguousarray(b_proj, dtype=np.float32),
        })
    res = bass_utils.run_bass_kernel_spmd(
        nc, in_maps, core_ids=list(range(N_CORES)), **run_kwargs
    )
    out = np.stack(
        [res.results[b]["out"].reshape(C, HH, WW) for b in range(B)], axis=0
    )
    _CACHE["last_result"] = res
    return out


# revision 35
# speedup vs baseline: 1.6366x; 1.3266x over previous
"""Trainium2 Bass kernel for an AttentionBlock (GroupNorm -> 1x1 qkv conv ->
multi-head self-attention -> 1x1 proj conv -> residual).

Full-input contract: kernel(**inputs) takes the complete unsharded inputs and
returns the complete output. Internally the batch dimension (B=8) is
data-parallel across the 8 NeuronCores: core b processes batch element b
end-to-end (no collectives needed).

Per-core layout strategy (all hardcoded for B=8, C=512, H=W=32 -> S=1024,
HEADS=8, D=64, GROUPS=32):
  - x is held channel-major in SBUF as [128 part, 4 blk, 1024 s].
  - GroupNorm: bn_stats/bn_aggr per channel, then tiny matmuls against a
    block-diagonal (1/16) matrix reduce the 16 channels of each group across
    partitions; rsqrt built as exp(-0.5*ln(var+eps)) so only one ACT table set
    (natural_log_exp) is ever loaded.
  - q,k are produced channel-major (needs W_qkv^T which is built once with PE
    transposes); v is produced directly spatial-major ([s, 64] per head) by
    contracting xn as lhsT, so no per-head transposes are needed.
  - Attention per head computes logits^T (s on partitions, t free) so softmax's
    normalization can ride the PV matmul: a ones-column appended to v^T yields
    the softmax denominators in PSUM row 64 for free. exp() runs on ScalarE
    straight out of PSUM with the 1/sqrt(D) scale folded into ACT's free
    affine; max-subtraction is skipped (logits are bounded ~ +-8, safe in f32).
  - Head pairs live in partition halves (base partition 0/64) so the K=64
    logits matmuls of a pair can run concurrently in separate PE row groups.
  - proj + bias + residual: matmul, then one DVE pass adding (x + b_proj).
"""

import numpy as np

B, C, S = 8, 512, 1024
HH, WW = 32, 32
HEADS, D, GROUPS, GS = 8, 64, 32, 16
EPS = 1e-5
P = 128
NCB = C // P  # 4 channel blocks
NSB = S // P  # 8 spatial blocks
N_CORES = 8

_CACHE = {}


def _build_program(debug_outs=False):
    import concourse.bacc as bacc
    import concourse.bass as bass
    import concourse.mybir as mybir
    import concourse.tile as tile
    from concourse.masks import make_identity

    f32 = mybir.dt.float32
    bf16 = mybir.dt.bfloat16
    AF = mybir.ActivationFunctionType
    OP = mybir.AluOpType

    nc = bacc.Bacc()

    x_d = nc.dram_tensor("x", [C, S], f32, kind="ExternalInput")
    gamma_d = nc.dram_tensor("gamma", [C], f32, kind="ExternalInput")
    beta_d = nc.dram_tensor("beta", [C], f32, kind="ExternalInput")
    wqkv_d = nc.dram_tensor("w_qkv", [3 * C, C], f32, kind="ExternalInput")
    bqkv_d = nc.dram_tensor("b_qkv", [3 * C], f32, kind="ExternalInput")
    wproj_d = nc.dram_tensor("w_proj", [C, C], f32, kind="ExternalInput")
    bproj_d = nc.dram_tensor("b_proj", [C], f32, kind="ExternalInput")
    out_d = nc.dram_tensor("out", [C, S], f32, kind="ExternalOutput")

    x_blk = x_d.rearrange("(blk p) s -> p blk s", p=P)        # [128, 4, 1024]
    out_blk = out_d.rearrange("(blk p) s -> p blk s", p=P)    # [128, 4, 1024]

    with tile.TileContext(nc) as tc:
        with (
            tc.tile_pool(name="big", bufs=1) as big,
            tc.tile_pool(name="wstage", bufs=4) as wstage,
            tc.tile_pool(name="small", bufs=4) as small,
            tc.tile_pool(name="exps", bufs=2) as exps,
            tc.tile_pool(name="psl", bufs=2, space="PSUM") as psl,
            tc.tile_pool(name="pso", bufs=2, space="PSUM") as pso,
            tc.tile_pool(name="psq", bufs=1, space="PSUM") as psq,
            tc.tile_pool(name="dpool", bufs=2, space="DRAM") as dpool,
        ):
            # PSUM budget (8 banks): psl 2x[128,1024] (logits, GN) = 4,
            # pso 2x[128,512] (PV/vT/transpose/proj) = 2, psq 1x[128,1024]
            # (qk, runs independently of the attention pipeline) = 2.
            # ---------------- input DMAs ----------------
            x_sb = big.tile([P, NCB, S], f32, tag="x_sb")
            for blk in range(NCB):
                nc.sync.dma_start(out=x_sb[:, blk, :], in_=x_blk[:, blk, :])

            gamma_sb = big.tile([P, NCB], f32, tag="gamma_sb")
            nc.sync.dma_start(out=gamma_sb, in_=gamma_d.rearrange("(blk p) -> p blk", p=P))
            beta_sb = big.tile([P, NCB], f32, tag="beta_sb")
            nc.sync.dma_start(out=beta_sb, in_=beta_d.rearrange("(blk p) -> p blk", p=P))
            bqkv_sb = big.tile([P, 12], f32, tag="bqkv_sb")
            nc.sync.dma_start(out=bqkv_sb, in_=bqkv_d.rearrange("(blk p) -> p blk", p=P))
            bproj_sb = big.tile([P, NCB], f32, tag="bproj_sb")
            nc.sync.dma_start(out=bproj_sb, in_=bproj_d.rearrange("(blk p) -> p blk", p=P))
            # b_v replicated across all partitions (DMA can broadcast, DVE can't)
            bv_rep = big.tile([P, HEADS, D], f32, tag="bv_rep")
            bv_src = bqkv_d[2 * C:3 * C]
            nc.sync.dma_start(
                out=bv_rep,
                in_=bass.AP(
                    tensor=bv_src.tensor,
                    offset=bv_src.offset,
                    ap=[[0, P], [D, HEADS], [1, D]],
                ),
            )

            # ---------------- constants ----------------
            ident = big.tile([P, P], f32, tag="ident")
            make_identity(nc, ident)
            eps_sb = big.tile([P, 1], f32, tag="eps_sb")
            nc.gpsimd.memset(eps_sb, EPS)
            # gmat[c, g] = 1/(16*S) if channel c (within a 128-block) is in
            # local group g else 0; gmat2 = indicator transpose (broadcast
            # back). Built with affine_select (out = cond ? in : fill) since
            # memsets on 16-partition slices violate the 32-align rule.
            gmat = big.tile([P, 8], f32, tag="gmat")
            nc.gpsimd.memset(gmat, 0.0)
            nc.gpsimd.affine_select(  # fill 1/16 where c-16g-15 <= 0
                out=gmat, in_=gmat, compare_op=OP.is_gt,
                fill=1.0 / GS, base=-15, pattern=[[-GS, 8]],
                channel_multiplier=1,
            )
            nc.gpsimd.affine_select(  # zero where c-16g < 0
                out=gmat, in_=gmat, compare_op=OP.is_ge,
                fill=0.0, base=0, pattern=[[-GS, 8]],
                channel_multiplier=1,
            )
            gmat2 = big.tile([8, P], f32, tag="gmat2")
            nc.gpsimd.memset(gmat2, 0.0)
            nc.gpsimd.affine_select(
                out=gmat2, in_=gmat2, compare_op=OP.is_gt,
                fill=1.0, base=-15, pattern=[[1, P]],
                channel_multiplier=-GS,
            )
            nc.gpsimd.affine_select(
                out=gmat2, in_=gmat2, compare_op=OP.is_ge,
                fill=0.0, base=0, pattern=[[1, P]],
                channel_multiplier=-GS,
            )

            # ---------------- W^T via PE transposes ----------------
            # stored bf16: fp32 matmuls run fp32_mode=LOW_HIGH (two passes,
            # ~2x slower), bf16 operands stream at full rate
            wqkvT = big.tile([P, NCB, 3 * C], bf16, tag="wqkvT")  # [i, iblk, o]
            for m in range(12):  # source row blocks of w_qkv (out channels)
                w_raw = wstage.tile([P, C], f32, tag="w_raw")
                nc.sync.dma_start(
                    out=w_raw,
                    in_=wqkv_d.rearrange("(mb p) i -> p mb i", p=P)[:, m, :],
                )
                for dblk in range(NCB):
                    pt = pso.tile([P, 512], f32, tag="pso")
                    nc.tensor.transpose(pt[:, 0:P], w_raw[:, dblk * P:(dblk + 1) * P], ident)
                    nc.vector.tensor_copy(
                        out=wqkvT[:, dblk, m * P:(m + 1) * P], in_=pt[:, 0:P]
                    )
            wprojT = big.tile([P, NCB, C], bf16, tag="wprojT")
            for m in range(NCB):
                w_raw = wstage.tile([P, C], f32, tag="w_raw")
                nc.sync.dma_start(
                    out=w_raw,
                    in_=wproj_d.rearrange("(mb p) i -> p mb i", p=P)[:, m, :],
                )
                for dblk in range(NCB):
                    pt = pso.tile([P, 512], f32, tag="pso")
                    nc.tensor.transpose(pt[:, 0:P], w_raw[:, dblk * P:(dblk + 1) * P], ident)
                    nc.vector.tensor_copy(
                        out=wprojT[:, dblk, m * P:(m + 1) * P], in_=pt[:, 0:P]
                    )

            # ---------------- GroupNorm ----------------
            # phase 1: per-channel stats + cross-partition group combine
            xn = big.tile([P, NCB, S], bf16, tag="xn")
            musrs, vargs = [], []
            for blk in range(NCB):
                stats = small.tile([P, 2, 6], f32, tag="stats")
                nc.vector.bn_stats(out=stats[:, 0, :], in_=x_sb[:, blk, 0:512])
                nc.vector.bn_stats(out=stats[:, 1, :], in_=x_sb[:, blk, 512:1024])
                mv = small.tile([P, 2], f32, tag="mv")
                nc.vector.bn_aggr(out=mv, in_=stats)
                # t2 = [mean, var + mean^2] per channel
                t2 = small.tile([P, 2], f32, tag="t2")
                nc.vector.tensor_copy(t2[:, 0:1], mv[:, 0:1])
                sq = small.tile([P, 1], f32, tag="sq")
                nc.vector.tensor_mul(sq, mv[:, 0:1], mv[:, 0:1])
                nc.vector.tensor_add(t2[:, 1:2], mv[:, 1:2], sq)
                # cross-partition group reduce: psg[g, :] = [mu_g, E2_g]
                # (bn stats are already per-channel means over S, so the
                #  group combine just averages the 16 channels: gmat = 1/16)
                psg = psl.tile([P, 1024], f32, tag="psl")
                nc.tensor.matmul(psg[0:8, 0:2], gmat, t2, start=True, stop=True)
                # var_g = E2 - mu^2 ; rstd = exp(-0.5*ln(var+eps))
                musr = small.tile([8, 2], f32, tag=f"musr{blk}")
                nc.vector.tensor_copy(musr, psg[0:8, 0:2])  # [mu, E2]
                sqmu = small.tile([8, 1], f32, tag="sqmu")
                nc.vector.tensor_mul(sqmu, musr[:, 0:1], musr[:, 0:1])
                varg = small.tile([8, 1], f32, tag=f"varg{blk}")
                nc.vector.tensor_tensor(varg, musr[:, 1:2], sqmu, OP.subtract)
                musrs.append(musr)
                vargs.append(varg)
            # phase 2: rstd = 1/sqrt(var+eps). Sqrt on ACT (its table set loads
            # once; Ln/Exp alternation would thrash table loads), exact
            # reciprocal on DVE ([8,1] is only 8 elements -> ~0.2us).
            for blk in range(NCB):
                sd = small.tile([8, 1], f32, tag=f"sd{blk}")
                nc.scalar.activation(sd, vargs[blk], AF.Sqrt, bias=eps_sb[0:8, :],
                                     scale=1.0)
                nc.vector.reciprocal(musrs[blk][:, 1:2], sd)
            # phase 3: broadcast (mu, rstd) back to channels and normalize
            for blk in range(NCB):
                psb = psl.tile([P, 1024], f32, tag="psl")
                nc.tensor.matmul(psb[:, 0:2], gmat2, musrs[blk], start=True,
                                 stop=True)
                scale_c = small.tile([P, 1], f32, tag="scale_c")
                nc.vector.tensor_mul(scale_c, psb[:, 1:2], gamma_sb[:, blk:blk + 1])
                nm = small.tile([P, 1], f32, tag="nm")
                nc.vector.tensor_mul(nm, psb[:, 0:1], scale_c)
                bias_c = small.tile([P, 1], f32, tag="bias_c")
                nc.vector.tensor_tensor(bias_c, beta_sb[:, blk:blk + 1], nm, OP.subtract)
                nc.vector.tensor_scalar(
                    xn[:, blk, :], x_sb[:, blk, :], scale_c, bias_c, OP.mult, OP.add
                )

            # xpb = x + b_proj (residual + proj bias), in place over x_sb
            for blk in range(NCB):
                nc.vector.tensor_scalar(
                    x_sb[:, blk, :], x_sb[:, blk, :], bproj_sb[:, blk:blk + 1], None,
                    OP.add,
                )

            # ---------------- v^T (spatial-major, with ones column) --------
            vT = big.tile([P, NSB, HEADS, D + 1], bf16, tag="vT")
            for sblk in range(NSB):
                pv = pso.tile([P, 512], f32, tag="pso")
                for kk in range(NCB):
                    nc.tensor.matmul(
                        pv,
                        xn[:, kk, sblk * P:(sblk + 1) * P],
                        wqkvT[:, kk, 2 * C:3 * C],
                        start=(kk == 0), stop=(kk == NCB - 1),
                    )
                nc.vector.tensor_tensor(
                    vT[:, sblk, :, 0:D],
                    pv.rearrange("p (h d) -> p h d", h=HEADS),
                    bv_rep,
                    OP.add,
                )
                nc.gpsimd.memset(vT[:, sblk, :, D:D + 1], 1.0)

            # ---------------- qk (channel-major) ----------------
            # qk_sb blocks 0..3 = q channels 0..511, blocks 4..7 = k channels.
            # Emitted interleaved with the attention pairs (next pair's qk
            # right after this pair's logits) so PE has exp-independent work
            # during ACT-gated gaps; psq is qk's dedicated PSUM slot.
            qk_sb = big.tile([P, 8, S], bf16, tag="qk_sb")

            def emit_qk(m):
                pq = psq.tile([P, 1024], f32, tag="psq")
                for n in range(2):
                    for kk in range(NCB):
                        nc.tensor.matmul(
                            pq[:, n * 512:(n + 1) * 512],
                            wqkvT[:, kk, m * P:(m + 1) * P],
                            xn[:, kk, n * 512:(n + 1) * 512],
                            start=(kk == 0), stop=(kk == NCB - 1),
                        )
                nc.vector.tensor_scalar(
                    qk_sb[:, m, :], pq, bqkv_sb[:, m:m + 1], None, OP.add
                )

            emit_qk(0)
            emit_qk(4)

            # ---------------- attention ----------------
            hout = big.tile([P, NCB, S], bf16, tag="hout")
            s2 = 1.0 / float(np.sqrt(D))  # folded 1/sqrt(sqrt(D)) on q and k
            for hp in range(4):  # head pairs -> PE row-group concurrency
                explts = []
                for sblk in range(NSB):
                    pls = []
                    for h2 in range(2):
                        h = 2 * hp + h2
                        p0 = (h % 2) * D
                        if sblk == 0:
                            explts.append(exps.tile([P, NSB, S], bf16,
                                                    name=f"explt{hp}_{h2}",
                                                    tag=f"explt{h2}"))
                        pl = psl.tile([P, 1024], f32, tag="psl")
                        pls.append(pl)
                        for n in range(2):
                            nc.tensor.matmul(
                                pl[:, n * 512:(n + 1) * 512],
                                qk_sb[p0:p0 + D, 4 + h // 2, sblk * P:(sblk + 1) * P],
                                qk_sb[p0:p0 + D, h // 2, n * 512:(n + 1) * 512],
                                start=True, stop=True,
                            )
                    for h2 in range(2):
                        nc.scalar.activation(
                            explts[h2][:, sblk, :], pls[h2], AF.Exp, scale=s2
                        )
                if hp < 3:  # next pair's qk fills PE gaps during this pair
                    emit_qk(hp + 1)
                    emit_qk(5 + hp)
                for h2 in range(2):
                    h = 2 * hp + h2
                    explt = explts[h2]
                    pos = []
                    rinv = small.tile([1, S], f32, tag="rinv")
                    for n in range(2):
                        po = pso.tile([P, 512], f32, tag="pso")
                        pos.append(po)
                        for sblk in range(NSB):
                            nc.tensor.matmul(
                                po[0:D + 1, :],
                                vT[:, sblk, h, :],
                                explt[:, sblk, n * 512:(n + 1) * 512],
                                start=(sblk == 0), stop=(sblk == NSB - 1),
                            )
                        # softmax denominator -> ~51-ULP reciprocal (exact
                        # DVE reciprocal is 8 cyc/elem on one lane = 6.5us).
                        # Copy to SBUF first: the custom-DVE approx op read
                        # from a base-partition-64 PSUM row returns garbage.
                        den = small.tile([1, 512], f32, tag="den")
                        nc.vector.tensor_copy(den, po[D:D + 1, :])
                        nc.vector.reciprocal_approx_fast(
                            rinv[:, n * 512:(n + 1) * 512], den
                        )
                    # replicate rinv across 64 partitions via a DRAM bounce
                    # (neither DVE nor SBUF-source DMA can partition-broadcast)
                    rdram = dpool.tile([S], f32, tag="rdram")
                    nc.sync.dma_start(out=rdram[None, :], in_=rinv)
                    rinv_rep = small.tile([D, S], f32, tag="rinv_rep")
                    nc.sync.dma_start(
                        out=rinv_rep, in_=rdram[None, :].partition_broadcast(D)
                    )
                    for n in range(2):
                        nc.vector.tensor_tensor(
                            hout[(h % 2) * D:(h % 2) * D + D, h // 2,
                                 n * 512:(n + 1) * 512],
                            pos[n][0:D, :],
                            rinv_rep[:, n * 512:(n + 1) * 512],
                            OP.mult,
                        )

            if debug_outs:
                dbg = {
                    "dbg_xn": xn, "dbg_qk": qk_sb,
                    "dbg_vt": vT, "dbg_hout": hout,
                }
                for name, tile_ap in dbg.items():
                    od = nc.dram_tensor(name, list(tile_ap.shape),
                                        tile_ap.dtype, kind="ExternalOutput")
                    nc.sync.dma_start(out=od[:], in_=tile_ap[:])

            # ---------------- proj + residual ----------------
            out_sb = big.tile([P, NCB, S], f32, tag="out_sb")
            for m in range(NCB):
                for n in range(2):
                    pp = pso.tile([P, 512], f32, tag="pso")
                    for kk in range(NCB):
                        nc.tensor.matmul(
                            pp,
                            wprojT[:, kk, m * P:(m + 1) * P],
                            hout[:, kk, n * 512:(n + 1) * 512],
                            start=(kk == 0), stop=(kk == NCB - 1),
                        )
                    nc.vector.tensor_tensor(
                        out_sb[:, m, n * 512:(n + 1) * 512], pp,
                        x_sb[:, m, n * 512:(n + 1) * 512], OP.add,
                    )
                nc.sync.dma_start(out=out_blk[:, m, :], in_=out_sb[:, m, :])

    # Bacc defers register allocation etc. to its compile pass; the
    # run_bass_via_pjrt path serializes the module as-is, so run it here.
    nc.finalize()
    return nc


def _get_program():
    if "nc" not in _CACHE:
        _CACHE["nc"] = _build_program()
    return _CACHE["nc"]


def kernel(x, gamma, beta, w_qkv, b_qkv, w_proj, b_proj, **run_kwargs):
    from concourse import bass_utils

    nc = _get_program()
    in_maps = []
    for b in range(B):
        in_maps.append({
            "x": np.ascontiguousarray(x[b].reshape(C, S), dtype=np.float32),
            "gamma": np.ascontiguousarray(gamma, dtype=np.float32),
            "beta": np.ascontiguousarray(beta, dtype=np.float32),
            "w_qkv": np.ascontiguousarray(w_qkv, dtype=np.float32),
            "b_qkv": np.ascontiguousarray(b_qkv, dtype=np.float32),
            "w_proj": np.ascontiguousarray(w_proj, dtype=np.float32),
            "b_proj": np.ascontiguousarray(b_proj, dtype=np.float32),
        })
    res = bass_utils.run_bass_kernel_spmd(
        nc, in_maps, core_ids=list(range(N_CORES)), **run_kwargs
    )
    out = np.stack(
        [res.results[b]["out"].reshape(C, HH, WW) for b in range(B)], axis=0
    )
    _CACHE["last_result"] = res
    return out
